# revision 5
# baseline (speedup 1.0000x reference)
"""Trainium2 Bass kernel for nn_BitwiseHashing.

Computes out = tanh(mean_l(x) @ W.T + b) for x:[12,8192,1024] f32,
W:[64,1024], b:[64] -> out:[8192,64].

Strategy (data-parallel over 8 NeuronCores):
  - shard x along batch dim: 1024 rows per core (48 MiB each, streamed).
  - host pre-transposes W to wt = bf16(W.T / L) [1024,64]; bias as bf16 [1,64].
  - per 128-row block: stream 12 L-slices (contiguous 512 KiB DMAs) on the
    two HWDGE rings, reduce with a DVE add tree that casts to bf16 at the
    first level (f32+f32->bf16, then bf16 adds at 2x DVE rate), PE-transpose
    the 8 [128,128] d-chunks of the bf16 sum (bf16 transposes are ~4x
    cheaper than f32), one PSUM->SBUF copy on ACT, bf16 matmuls against wt
    accumulating in f32 PSUM [128,64] (bias pre-loaded via a C=1
    ones-matmul), tanh on ScalarE, y written out over the SWDGE queue so
    the x-stream HWDGE rings never stall behind output writes.
"""

import numpy as np

import concourse.bacc as bacc
import concourse.mybir as mybir
from concourse import tile
from concourse.masks import make_identity
from concourse.bass_utils import run_bass_kernel_spmd

L, B, D, K = 12, 8192, 1024, 64
NCORES = 8
BS = B // NCORES      # 1024 batch rows per core
P = 128               # partitions
NBLK = BS // P        # 8 row blocks per core
NDC = D // P          # 8 contraction chunks
F32 = mybir.dt.float32
BF16 = mybir.dt.bfloat16

_nc_cache = None


def _build():
    global _nc_cache
    if _nc_cache is not None:
        return _nc_cache

    nc = bacc.Bacc("TRN2", target_bir_lowering=False, debug=False)
    x = nc.dram_tensor("x", [L, BS, D], F32, kind="ExternalInput")
    wt = nc.dram_tensor("wt", [D, K], BF16, kind="ExternalInput")
    bias = nc.dram_tensor("bias", [1, K], BF16, kind="ExternalInput")
    y = nc.dram_tensor("y", [BS, K], F32, kind="ExternalOutput")

    with tile.TileContext(nc) as tc:
        with (
            tc.tile_pool(name="const", bufs=1) as cpool,
            tc.tile_pool(name="xin", bufs=34) as xpool,
            tc.tile_pool(name="part", bufs=14) as ppool,
            tc.tile_pool(name="xt", bufs=2) as tpool,
            tc.tile_pool(name="out", bufs=3) as opool,
            tc.tile_pool(name="pt", bufs=2, space="PSUM") as pt_pool,
            tc.tile_pool(name="po", bufs=2, space="PSUM") as po_pool,
        ):
            # constants go over the SWDGE queue to keep both HWDGE rings
            # free for the x stream from t=0
            wt_sb = cpool.tile([P, NDC * K], BF16)
            for dc in range(NDC):
                nc.gpsimd.dma_start(
                    out=wt_sb[:, dc * K:(dc + 1) * K],
                    in_=wt.ap()[dc * P:(dc + 1) * P, :],
                )
            bias_sb = cpool.tile([1, K], BF16)
            nc.gpsimd.dma_start(out=bias_sb[:], in_=bias.ap())
            ones_sb = cpool.tile([1, P], BF16)
            nc.gpsimd.memset(ones_sb[:], 1.0)
            ident = cpool.tile([P, P], BF16)
            make_identity(nc, ident[:])

            xap = x.ap()
            yap = y.ap()

            def issue_loads(blk):
                b0 = blk * P
                xt = []
                for l in range(L):
                    xl = xpool.tile([P, D], F32)
                    eng = nc.sync if l % 2 == 0 else nc.scalar
                    eng.dma_start(out=xl[:], in_=xap[l, b0:b0 + P, :])
                    xt.append(xl)
                return xt

            def reduce(xt):
                # pair adds WITHIN each DMA ring (even tiles on the sync
                # ring, odd on the scalar ring complete in FIFO order within
                # their ring, so neither chain waits on cross-ring skew),
                # casting f32 -> bf16 at first touch so merges run at 2x DVE
                # rate. Ordered so that only ~2.6us of DVE work remains after
                # a block's LAST tiles land: the early pairs and their merge
                # run while the tail of the block is still streaming in.
                def add(i0, i1):
                    t = ppool.tile([P, D], BF16)
                    nc.vector.tensor_add(out=t[:], in0=i0[:], in1=i1[:])
                    return t

                ae = add(xt[0], xt[2])
                ao = add(xt[1], xt[3])
                be = add(xt[4], xt[6])
                bo = add(xt[5], xt[7])
                ce = add(ae, be)
                co = add(ao, bo)
                ee = add(xt[8], xt[10])
                eo = add(xt[9], xt[11])
                fe = add(ce, ee)
                fo = add(co, eo)
                return add(fe, fo)

            def project(acc):
                # transpose the bf16 block sum into PSUM (single-op groups),
                # then per-chunk PSUM->SBUF copies on ACT: copy chunk dc only
                # waits on transpose dc, so the scalar engine (which also
                # triggers half the x stream) resumes ~2.5us earlier than
                # with one whole-tile copy waiting on all 8 transposes
                pt_all = pt_pool.tile([P, D], BF16)
                for dc in range(NDC):
                    nc.tensor.transpose(
                        pt_all[:, dc * P:(dc + 1) * P],
                        acc[:, dc * P:(dc + 1) * P],
                        ident[:],
                    )
                xt_all = tpool.tile([P, D], BF16)
                for dc in range(NDC):
                    nc.scalar.copy(
                        out=xt_all[:, dc * P:(dc + 1) * P],
                        in_=pt_all[:, dc * P:(dc + 1) * P],
                    )

                po = po_pool.tile([P, K], F32)
                # bias broadcast across partitions: ones[1,128].T @ bias[1,64]
                nc.tensor.matmul(
                    po[:], lhsT=ones_sb[:], rhs=bias_sb[:], start=True, stop=False
                )
                for dc in range(NDC):
                    nc.tensor.matmul(
                        po[:],
                        lhsT=xt_all[:, dc * P:(dc + 1) * P],
                        rhs=wt_sb[:, dc * K:(dc + 1) * K],
                        start=False,
                        stop=(dc == NDC - 1),
                    )
                return po

            def finish(blk, po):
                b0 = blk * P
                ot = opool.tile([P, K], F32)
                nc.scalar.activation(
                    ot[:], po[:], mybir.ActivationFunctionType.Tanh
                )
                # y goes over the SWDGE queue: the HWDGE rings carry only the
                # x stream, so a pending tanh can never stall x loads queued
                # behind a y write in ring FIFO order
                nc.gpsimd.dma_start(out=yap[b0:b0 + P, :], in_=ot[:])

            # Emission order per block: adds(n) -> loads(n+1) -> psum/matmul
            # stage(n) -> tanh+y(n-1). This keeps every ACT/sync DMA trigger
            # for block n+1 AHEAD of block n's copy/tanh in the engine
            # FIFOs, so the two x-stream rings never stall behind compute.
            xt = issue_loads(0)
            prev_po = None
            for blk in range(NBLK):
                acc = reduce(xt)
                if blk + 1 < NBLK:
                    xt = issue_loads(blk + 1)
                po = project(acc)
                if prev_po is not None:
                    finish(blk - 1, prev_po)
                prev_po = po
            finish(NBLK - 1, prev_po)

    nc.compile()
    _nc_cache = nc
    return nc


def _ensure_ntff_hook():
    """Register the axon NTFF profile hook if the image's antenv lacks it."""
    import sys
    import types

    try:
        from antenv.axon_hooks import get_axon_ntff_profile_hook  # noqa: F401
        return
    except ImportError:
        pass
    import antenv

    mod = types.ModuleType("antenv.axon_hooks")
    mod._hook = None

    def set_axon_ntff_profile_hook(h):
        mod._hook = h

    def get_axon_ntff_profile_hook():
        return mod._hook

    mod.set_axon_ntff_profile_hook = set_axon_ntff_profile_hook
    mod.get_axon_ntff_profile_hook = get_axon_ntff_profile_hook
    sys.modules["antenv.axon_hooks"] = mod
    antenv.axon_hooks = mod
    try:
        from trn_agent_boot.trn_boot import _ntff_profile_via_ctypes

        mod._hook = _ntff_profile_via_ctypes("/opt/axon/libaxon_pjrt.so")
    except Exception:
        mod._hook = None


def _run(inputs, trace=False, **kwargs):
    import ml_dtypes

    bf16 = np.dtype(ml_dtypes.bfloat16)
    x = np.asarray(inputs["x"], dtype=np.float32)
    W = np.asarray(inputs["W"], dtype=np.float32)
    b = np.asarray(inputs["b"], dtype=np.float32)
    wt = (np.ascontiguousarray(W.T) * np.float32(1.0 / L)).astype(bf16)
    bias = np.ascontiguousarray(b.reshape(1, K)).astype(bf16)
    in_maps = [
        {
            "x": np.ascontiguousarray(x[:, c * BS:(c + 1) * BS, :]),
            "wt": wt,
            "bias": bias,
        }
        for c in range(NCORES)
    ]
    if trace:
        _ensure_ntff_hook()
        import concourse.bass_utils as bu

        bu.upload_artifacts = lambda tmpdir: "local://skipped"
    nc = _build()
    res = run_bass_kernel_spmd(
        nc, in_maps, core_ids=list(range(NCORES)), trace=trace, **kwargs
    )
    y = np.concatenate([r["y"] for r in res.results], axis=0)
    return y, res


def kernel(**inputs):
    y, _ = _run(inputs)
    return y


# revision 8
# speedup vs baseline: 1.0204x; 1.0204x over previous
"""Trainium2 Bass kernel for nn_BitwiseHashing.

Computes out = tanh(mean_l(x) @ W.T + b) for x:[12,8192,1024] f32,
W:[64,1024], b:[64] -> out:[8192,64].

Strategy (data-parallel over 8 NeuronCores):
  - shard x along batch dim: 1024 rows per core (48 MiB each, streamed).
  - host pre-transposes W to wt = bf16(W.T / L) [1024,64]; bias as bf16 [1,64].
  - per 128-row block: stream 12 L-slices (contiguous 512 KiB DMAs) on the
    two HWDGE rings, reduce with a DVE add tree that casts to bf16 at the
    first level (f32+f32->bf16, then bf16 adds at 2x DVE rate), PE-transpose
    the 8 [128,128] d-chunks of the bf16 sum (bf16 transposes are ~4x
    cheaper than f32), one PSUM->SBUF copy on ACT, bf16 matmuls against wt
    accumulating in f32 PSUM [128,64] (bias pre-loaded via a C=1
    ones-matmul), tanh on ScalarE, y written out over the SWDGE queue so
    the x-stream HWDGE rings never stall behind output writes.
"""

import numpy as np

import concourse.bacc as bacc
import concourse.mybir as mybir
from concourse import tile
from concourse.masks import make_identity
from concourse.bass_utils import run_bass_kernel_spmd

L, B, D, K = 12, 8192, 1024, 64
NCORES = 8
BS = B // NCORES      # 1024 batch rows per core
P = 128               # partitions
NBLK = BS // P        # 8 row blocks per core
NDC = D // P          # 8 contraction chunks
F32 = mybir.dt.float32
BF16 = mybir.dt.bfloat16

_nc_cache = None


def _build():
    global _nc_cache
    if _nc_cache is not None:
        return _nc_cache

    nc = bacc.Bacc("TRN2", target_bir_lowering=False, debug=False)
    x = nc.dram_tensor("x", [L, BS, D], F32, kind="ExternalInput")
    wt = nc.dram_tensor("wt", [D, K], BF16, kind="ExternalInput")
    bias = nc.dram_tensor("bias", [1, K], BF16, kind="ExternalInput")
    y = nc.dram_tensor("y", [BS, K], F32, kind="ExternalOutput")

    with tile.TileContext(nc) as tc:
        with (
            tc.tile_pool(name="const", bufs=1) as cpool,
            tc.tile_pool(name="xin", bufs=34) as xpool,
            tc.tile_pool(name="part", bufs=16) as ppool,
            tc.tile_pool(name="xt", bufs=2) as tpool,
            tc.tile_pool(name="out", bufs=3) as opool,
            tc.tile_pool(name="pt", bufs=2, space="PSUM") as pt_pool,
            tc.tile_pool(name="po", bufs=2, space="PSUM") as po_pool,
        ):
            # constants go over the SWDGE queue to keep both HWDGE rings
            # free for the x stream from t=0
            wt_sb = cpool.tile([P, NDC * K], BF16)
            for dc in range(NDC):
                nc.gpsimd.dma_start(
                    out=wt_sb[:, dc * K:(dc + 1) * K],
                    in_=wt.ap()[dc * P:(dc + 1) * P, :],
                )
            bias_sb = cpool.tile([1, K], BF16)
            nc.gpsimd.dma_start(out=bias_sb[:], in_=bias.ap())
            ones_sb = cpool.tile([1, P], BF16)
            nc.gpsimd.memset(ones_sb[:], 1.0)
            ident = cpool.tile([P, P], BF16)
            make_identity(nc, ident[:])

            xap = x.ap()
            yap = y.ap()

            def issue_loads(blk):
                b0 = blk * P
                xt = []
                for l in range(L):
                    xl = xpool.tile([P, D], F32)
                    eng = nc.sync if l % 2 == 0 else nc.scalar
                    eng.dma_start(out=xl[:], in_=xap[l, b0:b0 + P, :])
                    xt.append(xl)
                return xt

            def reduce(xt, mid_cb=None):
                # pair adds WITHIN each DMA ring (even tiles on the sync
                # ring, odd on the scalar ring complete in FIFO order within
                # their ring, so neither chain waits on cross-ring skew),
                # casting f32 -> bf16 at first touch so merges run at 2x DVE
                # rate. mid_cb emits the PREVIOUS block's PSUM->SBUF copies
                # (on DVE) + matmuls here, after 4 adds: by then the previous
                # block's transposes are long done, so the copies never stall
                # the DVE instruction stream.
                def add(i0, i1):
                    t = ppool.tile([P, D], BF16)
                    nc.vector.tensor_add(out=t[:], in0=i0[:], in1=i1[:])
                    return t

                ae = add(xt[0], xt[2])
                ao = add(xt[1], xt[3])
                be = add(xt[4], xt[6])
                bo = add(xt[5], xt[7])
                if mid_cb is not None:
                    mid_cb()
                ce = add(ae, be)
                co = add(ao, bo)
                ee = add(xt[8], xt[10])
                eo = add(xt[9], xt[11])
                fe = add(ce, ee)
                fo = add(co, eo)
                return add(fe, fo)

            def transpose_block(acc):
                # transpose the bf16 block sum into PSUM (single-op groups)
                pt_all = pt_pool.tile([P, D], BF16)
                for dc in range(NDC):
                    nc.tensor.transpose(
                        pt_all[:, dc * P:(dc + 1) * P],
                        acc[:, dc * P:(dc + 1) * P],
                        ident[:],
                    )
                return pt_all

            def project_finish(blk, pt_all):
                # PSUM->SBUF chunk copies on DVE (cheap there, and emitted
                # mid-way through the NEXT block's adds so the transposes
                # they wait on are already done), then the K-projection on
                # PE, tanh on ACT, y out over the SWDGE queue so the HWDGE
                # rings carry nothing but the x stream
                xt_all = tpool.tile([P, D], BF16)
                for dc in range(NDC):
                    nc.vector.tensor_copy(
                        out=xt_all[:, dc * P:(dc + 1) * P],
                        in_=pt_all[:, dc * P:(dc + 1) * P],
                    )

                po = po_pool.tile([P, K], F32)
                # bias broadcast across partitions: ones[1,128].T @ bias[1,64]
                nc.tensor.matmul(
                    po[:], lhsT=ones_sb[:], rhs=bias_sb[:], start=True, stop=False
                )
                for dc in range(NDC):
                    nc.tensor.matmul(
                        po[:],
                        lhsT=xt_all[:, dc * P:(dc + 1) * P],
                        rhs=wt_sb[:, dc * K:(dc + 1) * K],
                        start=False,
                        stop=(dc == NDC - 1),
                    )
                ot = opool.tile([P, K], F32)
                nc.scalar.activation(
                    ot[:], po[:], mybir.ActivationFunctionType.Tanh
                )
                b0 = blk * P
                nc.gpsimd.dma_start(out=yap[b0:b0 + P, :], in_=ot[:])

            # Software pipeline per iteration blk:
            #   adds(blk) [DVE, with copies+matmuls+tanh of blk-1 emitted
            #   after the 4th add] -> loads(blk+1) triggers [sync/scalar] ->
            #   transposes(blk) [PE]. No engine's instruction stream ever
            #   waits across the block boundary: the rings see only triggers,
            #   DVE's mid-block detour only touches work that is already
            #   complete, and PE follows DVE by less than a block.
            xt = issue_loads(0)
            prev = None
            for blk in range(NBLK):
                if prev is None:
                    acc = reduce(xt)
                else:
                    pblk, ppt = prev
                    acc = reduce(xt, mid_cb=lambda: project_finish(pblk, ppt))
                if blk + 1 < NBLK:
                    xt = issue_loads(blk + 1)
                prev = (blk, transpose_block(acc))
            project_finish(prev[0], prev[1])

    nc.compile()
    _nc_cache = nc
    return nc


def _ensure_ntff_hook():
    """Register the axon NTFF profile hook if the image's antenv lacks it."""
    import sys
    import types

    try:
        from antenv.axon_hooks import get_axon_ntff_profile_hook  # noqa: F401
        return
    except ImportError:
        pass
    import antenv

    mod = types.ModuleType("antenv.axon_hooks")
    mod._hook = None

    def set_axon_ntff_profile_hook(h):
        mod._hook = h

    def get_axon_ntff_profile_hook():
        return mod._hook

    mod.set_axon_ntff_profile_hook = set_axon_ntff_profile_hook
    mod.get_axon_ntff_profile_hook = get_axon_ntff_profile_hook
    sys.modules["antenv.axon_hooks"] = mod
    antenv.axon_hooks = mod
    try:
        from trn_agent_boot.trn_boot import _ntff_profile_via_ctypes

        mod._hook = _ntff_profile_via_ctypes("/opt/axon/libaxon_pjrt.so")
    except Exception:
        mod._hook = None


def _run(inputs, trace=False, **kwargs):
    import ml_dtypes

    bf16 = np.dtype(ml_dtypes.bfloat16)
    x = np.asarray(inputs["x"], dtype=np.float32)
    W = np.asarray(inputs["W"], dtype=np.float32)
    b = np.asarray(inputs["b"], dtype=np.float32)
    wt = (np.ascontiguousarray(W.T) * np.float32(1.0 / L)).astype(bf16)
    bias = np.ascontiguousarray(b.reshape(1, K)).astype(bf16)
    in_maps = [
        {
            "x": np.ascontiguousarray(x[:, c * BS:(c + 1) * BS, :]),
            "wt": wt,
            "bias": bias,
        }
        for c in range(NCORES)
    ]
    if trace:
        _ensure_ntff_hook()
        import concourse.bass_utils as bu

        bu.upload_artifacts = lambda tmpdir: "local://skipped"
    nc = _build()
    res = run_bass_kernel_spmd(
        nc, in_maps, core_ids=list(range(NCORES)), trace=trace, **kwargs
    )
    y = np.concatenate([r["y"] for r in res.results], axis=0)
    return y, res


def kernel(**inputs):
    y, _ = _run(inputs)
    return y


# revision 13
# speedup vs baseline: 1.0378x; 1.0170x over previous
"""Trainium2 Bass kernel for nn_BitwiseHashing.

Computes out = tanh(mean_l(x) @ W.T + b) for x:[12,8192,1024] f32,
W:[64,1024], b:[64] -> out:[8192,64].

Strategy (data-parallel over 8 NeuronCores):
  - shard x along batch dim: 1024 rows per core (48 MiB each, streamed).
  - host pre-transposes W to wt = bf16(W.T / L) [1024,64]; bias as bf16 [1,64].
  - per 128-row block: stream 12 L-slices (contiguous 512 KiB DMAs) on the
    two HWDGE rings, reduce with a DVE add tree that casts to bf16 at the
    first level (f32+f32->bf16, then bf16 adds at 2x DVE rate), PE-transpose
    the 8 [128,128] d-chunks of the bf16 sum (bf16 transposes are ~4x
    cheaper than f32), one PSUM->SBUF copy on ACT, bf16 matmuls against wt
    accumulating in f32 PSUM [128,64] (bias pre-loaded via a C=1
    ones-matmul), tanh on ScalarE, y written out over the SWDGE queue so
    the x-stream HWDGE rings never stall behind output writes.
"""

import numpy as np

import concourse.bacc as bacc
import concourse.mybir as mybir
from concourse import tile
from concourse.masks import make_identity
from concourse.bass_utils import run_bass_kernel_spmd

L, B, D, K = 12, 8192, 1024, 64
NCORES = 8
BS = B // NCORES      # 1024 batch rows per core
P = 128               # partitions
NBLK = BS // P        # 8 row blocks per core
NDC = D // P          # 8 contraction chunks
F32 = mybir.dt.float32
BF16 = mybir.dt.bfloat16

_nc_cache = None


def _build():
    global _nc_cache
    if _nc_cache is not None:
        return _nc_cache

    nc = bacc.Bacc("TRN2", target_bir_lowering=False, debug=False)
    x = nc.dram_tensor("x", [L, BS, D], F32, kind="ExternalInput")
    wt = nc.dram_tensor("wt", [D, K], BF16, kind="ExternalInput")
    bias = nc.dram_tensor("bias", [1, K], BF16, kind="ExternalInput")
    y = nc.dram_tensor("y", [BS, K], F32, kind="ExternalOutput")

    with tile.TileContext(nc) as tc:
        with (
            tc.tile_pool(name="const", bufs=1) as cpool,
            tc.tile_pool(name="xin", bufs=36) as xpool,
            tc.tile_pool(name="part", bufs=16) as ppool,
            tc.tile_pool(name="xt", bufs=2) as tpool,
            tc.tile_pool(name="out", bufs=3) as opool,
            tc.tile_pool(name="pt", bufs=2, space="PSUM") as pt_pool,
            tc.tile_pool(name="po", bufs=2, space="PSUM") as po_pool,
        ):
            # constants go over the SWDGE queue to keep both HWDGE rings
            # free for the x stream from t=0
            wt_sb = cpool.tile([P, NDC * K], BF16)
            for dc in range(NDC):
                nc.gpsimd.dma_start(
                    out=wt_sb[:, dc * K:(dc + 1) * K],
                    in_=wt.ap()[dc * P:(dc + 1) * P, :],
                )
            bias_sb = cpool.tile([1, K], BF16)
            nc.gpsimd.dma_start(out=bias_sb[:], in_=bias.ap())
            ones_sb = cpool.tile([1, P], BF16)
            nc.gpsimd.memset(ones_sb[:], 1.0)
            ident = cpool.tile([P, P], BF16)
            make_identity(nc, ident[:])

            xap = x.ap()
            yap = y.ap()

            def issue_loads(blk):
                b0 = blk * P
                xt = []
                for l in range(L):
                    xl = xpool.tile([P, D], F32)
                    eng = nc.sync if l % 2 == 0 else nc.scalar
                    eng.dma_start(out=xl[:], in_=xap[l, b0:b0 + P, :])
                    xt.append(xl)
                return xt

            def reduce(xt, mid_cb=None):
                # pair adds WITHIN each DMA ring (even tiles on the sync
                # ring, odd on the scalar ring complete in FIFO order within
                # their ring, so neither chain waits on cross-ring skew),
                # casting f32 -> bf16 at first touch so merges run at 2x DVE
                # rate. mid_cb emits the PREVIOUS block's PSUM->SBUF copies
                # (on DVE) + matmuls here, after 4 adds: by then the previous
                # block's transposes are long done, so the copies never stall
                # the DVE instruction stream.
                def add(i0, i1):
                    t = ppool.tile([P, D], BF16)
                    nc.vector.tensor_add(out=t[:], in0=i0[:], in1=i1[:])
                    return t

                ae = add(xt[0], xt[2])
                ao = add(xt[1], xt[3])
                be = add(xt[4], xt[6])
                bo = add(xt[5], xt[7])
                if mid_cb is not None:
                    mid_cb()
                ce = add(ae, be)
                co = add(ao, bo)
                ee = add(xt[8], xt[10])
                eo = add(xt[9], xt[11])
                fe = add(ce, ee)
                fo = add(co, eo)
                return add(fe, fo)

            def transpose_block(acc):
                # transpose the bf16 block sum into PSUM (single-op groups)
                pt_all = pt_pool.tile([P, D], BF16)
                for dc in range(NDC):
                    nc.tensor.transpose(
                        pt_all[:, dc * P:(dc + 1) * P],
                        acc[:, dc * P:(dc + 1) * P],
                        ident[:],
                    )
                return pt_all

            def project_finish(blk, pt_all):
                # PSUM->SBUF chunk copies on DVE (cheap there, and emitted
                # mid-way through the NEXT block's adds so the transposes
                # they wait on are already done), then the K-projection on
                # PE, tanh on ACT, y out over the SWDGE queue so the HWDGE
                # rings carry nothing but the x stream
                xt_all = tpool.tile([P, D], BF16)
                for dc in range(NDC):
                    nc.vector.tensor_copy(
                        out=xt_all[:, dc * P:(dc + 1) * P],
                        in_=pt_all[:, dc * P:(dc + 1) * P],
                    )

                po = po_pool.tile([P, K], F32)
                # bias broadcast across partitions: ones[1,128].T @ bias[1,64]
                nc.tensor.matmul(
                    po[:], lhsT=ones_sb[:], rhs=bias_sb[:], start=True, stop=False
                )
                for dc in range(NDC):
                    nc.tensor.matmul(
                        po[:],
                        lhsT=xt_all[:, dc * P:(dc + 1) * P],
                        rhs=wt_sb[:, dc * K:(dc + 1) * K],
                        start=False,
                        stop=(dc == NDC - 1),
                    )
                ot = opool.tile([P, K], F32)
                nc.scalar.activation(
                    ot[:], po[:], mybir.ActivationFunctionType.Tanh
                )
                b0 = blk * P
                nc.gpsimd.dma_start(out=yap[b0:b0 + P, :], in_=ot[:])

            def tail_block(blk, xt, mid_cb):
                # last block: same arithmetic as reduce+transpose+project,
                # but the post-arrival chain is split into column halves so
                # DVE adds on the hi half overlap PE transposes on the lo
                # half, shaving ~3us off the kernel tail
                def addf(i0, i1):
                    t = ppool.tile([P, D], BF16)
                    nc.vector.tensor_add(out=t[:], in0=i0[:], in1=i1[:])
                    return t

                ae = addf(xt[0], xt[2])
                ao = addf(xt[1], xt[3])
                be = addf(xt[4], xt[6])
                bo = addf(xt[5], xt[7])
                mid_cb()
                ce = addf(ae, be)
                co = addf(ao, bo)

                H = D // 2
                NH = H // P
                pt_all = pt_pool.tile([P, D], BF16)
                xt_all = tpool.tile([P, D], BF16)
                po = po_pool.tile([P, K], F32)
                for h in range(2):
                    sl = slice(h * H, (h + 1) * H)

                    def addh(a0, s0, a1, s1):
                        t = ppool.tile([P, H], BF16)
                        nc.vector.tensor_add(out=t[:], in0=a0[s0], in1=a1[s1])
                        return t

                    cfull = (slice(None), sl)
                    hfull = (slice(None), slice(None))
                    eh = addh(xt[8], cfull, xt[10], cfull)
                    oh = addh(xt[9], cfull, xt[11], cfull)
                    fh = addh(ce, cfull, eh, hfull)
                    gh = addh(co, cfull, oh, hfull)
                    ah = addh(fh, hfull, gh, hfull)
                    for dcl in range(NH):
                        dc = h * NH + dcl
                        nc.tensor.transpose(
                            pt_all[:, dc * P:(dc + 1) * P],
                            ah[:, dcl * P:(dcl + 1) * P],
                            ident[:],
                        )
                nc.tensor.matmul(
                    po[:], lhsT=ones_sb[:], rhs=bias_sb[:], start=True, stop=False
                )
                for h in range(2):
                    for dcl in range(NH):
                        dc = h * NH + dcl
                        nc.vector.tensor_copy(
                            out=xt_all[:, dc * P:(dc + 1) * P],
                            in_=pt_all[:, dc * P:(dc + 1) * P],
                        )
                    for dcl in range(NH):
                        dc = h * NH + dcl
                        nc.tensor.matmul(
                            po[:],
                            lhsT=xt_all[:, dc * P:(dc + 1) * P],
                            rhs=wt_sb[:, dc * K:(dc + 1) * K],
                            start=False,
                            stop=(dc == NDC - 1),
                        )
                ot = opool.tile([P, K], F32)
                nc.scalar.activation(
                    ot[:], po[:], mybir.ActivationFunctionType.Tanh
                )
                # rings are idle by now; HWDGE has lower first-byte latency
                # than SWDGE, so the final y write goes out over sync
                b0 = blk * P
                nc.sync.dma_start(out=yap[b0:b0 + P, :], in_=ot[:])

            # Software pipeline per iteration blk:
            #   adds(blk) [DVE, with copies+matmuls+tanh of blk-1 emitted
            #   after the 4th add] -> loads(blk+1) triggers [sync/scalar] ->
            #   transposes(blk) [PE]. No engine's instruction stream ever
            #   waits across the block boundary: the rings see only triggers,
            #   DVE's mid-block detour only touches work that is already
            #   complete, and PE follows DVE by less than a block.
            xt = issue_loads(0)
            prev = None
            for blk in range(NBLK - 1):
                if prev is None:
                    acc = reduce(xt)
                else:
                    pblk, ppt = prev
                    acc = reduce(xt, mid_cb=lambda: project_finish(pblk, ppt))
                xt = issue_loads(blk + 1)
                prev = (blk, transpose_block(acc))
            pblk, ppt = prev
            tail_block(NBLK - 1, xt, mid_cb=lambda: project_finish(pblk, ppt))

    nc.compile()
    _nc_cache = nc
    return nc


def _ensure_ntff_hook():
    """Register the axon NTFF profile hook if the image's antenv lacks it."""
    import sys
    import types

    try:
        from antenv.axon_hooks import get_axon_ntff_profile_hook  # noqa: F401
        return
    except ImportError:
        pass
    import antenv

    mod = types.ModuleType("antenv.axon_hooks")
    mod._hook = None

    def set_axon_ntff_profile_hook(h):
        mod._hook = h

    def get_axon_ntff_profile_hook():
        return mod._hook

    mod.set_axon_ntff_profile_hook = set_axon_ntff_profile_hook
    mod.get_axon_ntff_profile_hook = get_axon_ntff_profile_hook
    sys.modules["antenv.axon_hooks"] = mod
    antenv.axon_hooks = mod
    try:
        from trn_agent_boot.trn_boot import _ntff_profile_via_ctypes

        mod._hook = _ntff_profile_via_ctypes("/opt/axon/libaxon_pjrt.so")
    except Exception:
        mod._hook = None


def _run(inputs, trace=False, **kwargs):
    import ml_dtypes

    bf16 = np.dtype(ml_dtypes.bfloat16)
    x = np.asarray(inputs["x"], dtype=np.float32)
    W = np.asarray(inputs["W"], dtype=np.float32)
    b = np.asarray(inputs["b"], dtype=np.float32)
    wt = (np.ascontiguousarray(W.T) * np.float32(1.0 / L)).astype(bf16)
    bias = np.ascontiguousarray(b.reshape(1, K)).astype(bf16)
    in_maps = [
        {
            "x": np.ascontiguousarray(x[:, c * BS:(c + 1) * BS, :]),
            "wt": wt,
            "bias": bias,
        }
        for c in range(NCORES)
    ]
    if trace:
        _ensure_ntff_hook()
        import concourse.bass_utils as bu

        bu.upload_artifacts = lambda tmpdir: "local://skipped"
    nc = _build()
    res = run_bass_kernel_spmd(
        nc, in_maps, core_ids=list(range(NCORES)), trace=trace, **kwargs
    )
    y = np.concatenate([r["y"] for r in res.results], axis=0)
    return y, res


def kernel(**inputs):
    y, _ = _run(inputs)
    return y


# revision 17
# speedup vs baseline: 1.2201x; 1.1757x over previous
"""Trainium2 Bass kernel for nn_BitwiseHashing.

Computes out = tanh(mean_l(x) @ W.T + b) for x:[12,8192,1024] f32,
W:[64,1024], b:[64] -> out:[8192,64].

Strategy (data-parallel over 8 NeuronCores):
  - shard x along batch dim: 1024 rows per core (48 MiB each, streamed).
  - host pre-transposes W to wt = bf16(W.T / L) [1024,64]; bias as bf16 [1,64].
  - per 128-row block: stream 12 L-slices (contiguous 512 KiB DMAs) on the
    two HWDGE rings, reduce with a DVE add tree that casts to bf16 at the
    first level (f32+f32->bf16, then bf16 adds at 2x DVE rate), PE-transpose
    the 8 [128,128] d-chunks of the bf16 sum (bf16 transposes are ~4x
    cheaper than f32), PSUM->SBUF chunk copies on DVE, bf16 matmuls against
    wt accumulating in f32 PSUM [128,64] (bias pre-loaded via a C=1
    ones-matmul), tanh on ScalarE.
  - software pipeline keeps both HWDGE rings fed with nothing but x-load
    triggers: the previous block's copies/matmuls/tanh are emitted mid-way
    through the next block's add sequence (their deps are complete by then),
    and y writes sit behind the next block's triggers in the sync FIFO.
    The final block is processed in column halves to overlap its DVE tail
    with PE transposes, shortening the kernel tail.
"""

import numpy as np

import concourse.bacc as bacc
import concourse.mybir as mybir
from concourse import tile
from concourse.masks import make_identity
from concourse.bass_utils import run_bass_kernel_spmd

L, B, D, K = 12, 8192, 1024, 64
NCORES = 8
BS = B // NCORES      # 1024 batch rows per core
P = 128               # partitions
NBLK = BS // P        # 8 row blocks per core
NDC = D // P          # 8 contraction chunks
F32 = mybir.dt.float32
BF16 = mybir.dt.bfloat16

_nc_cache = None


def _build():
    global _nc_cache
    if _nc_cache is not None:
        return _nc_cache

    nc = bacc.Bacc("TRN2", target_bir_lowering=False, debug=False)
    x = nc.dram_tensor("x", [L, BS, D], F32, kind="ExternalInput")
    wt = nc.dram_tensor("wt", [D, K], BF16, kind="ExternalInput")
    bias = nc.dram_tensor("bias", [1, K], BF16, kind="ExternalInput")
    y = nc.dram_tensor("y", [BS, K], F32, kind="ExternalOutput")

    with tile.TileContext(nc) as tc:
        with (
            tc.tile_pool(name="const", bufs=1) as cpool,
            tc.tile_pool(name="xin", bufs=36) as xpool,
            tc.tile_pool(name="part", bufs=16) as ppool,
            tc.tile_pool(name="xt", bufs=2) as tpool,
            tc.tile_pool(name="out", bufs=3) as opool,
            tc.tile_pool(name="pt", bufs=2, space="PSUM") as pt_pool,
            tc.tile_pool(name="po", bufs=2, space="PSUM") as po_pool,
        ):
            # constants go over the SWDGE queue to keep both HWDGE rings
            # free for the x stream from t=0
            wt_sb = cpool.tile([P, NDC * K], BF16)
            for dc in range(NDC):
                nc.gpsimd.dma_start(
                    out=wt_sb[:, dc * K:(dc + 1) * K],
                    in_=wt.ap()[dc * P:(dc + 1) * P, :],
                )
            bias_sb = cpool.tile([1, K], BF16)
            nc.gpsimd.dma_start(out=bias_sb[:], in_=bias.ap())
            ones_sb = cpool.tile([1, P], BF16)
            nc.gpsimd.memset(ones_sb[:], 1.0)
            ident = cpool.tile([P, P], BF16)
            make_identity(nc, ident[:])

            xap = x.ap()
            yap = y.ap()

            def issue_loads(blk):
                b0 = blk * P
                xt = []
                for l in range(L):
                    xl = xpool.tile([P, D], F32)
                    eng = nc.sync if l % 2 == 0 else nc.scalar
                    eng.dma_start(out=xl[:], in_=xap[l, b0:b0 + P, :])
                    xt.append(xl)
                return xt

            def reduce(xt, mid_cb=None):
                # pair adds WITHIN each DMA ring (even tiles on the sync
                # ring, odd on the scalar ring complete in FIFO order within
                # their ring, so neither chain waits on cross-ring skew),
                # casting f32 -> bf16 at first touch so merges run at 2x DVE
                # rate. mid_cb emits the PREVIOUS block's PSUM->SBUF copies
                # (on DVE) + matmuls here, after 4 adds: by then the previous
                # block's transposes are long done, so the copies never stall
                # the DVE instruction stream.
                def add(i0, i1):
                    t = ppool.tile([P, D], BF16)
                    nc.vector.tensor_add(out=t[:], in0=i0[:], in1=i1[:])
                    return t

                ae = add(xt[0], xt[2])
                ao = add(xt[1], xt[3])
                be = add(xt[4], xt[6])
                bo = add(xt[5], xt[7])
                if mid_cb is not None:
                    mid_cb()
                ce = add(ae, be)
                co = add(ao, bo)
                ee = add(xt[8], xt[10])
                eo = add(xt[9], xt[11])
                fe = add(ce, ee)
                fo = add(co, eo)
                return add(fe, fo)

            def transpose_block(acc):
                # transpose the bf16 block sum into PSUM (single-op groups)
                pt_all = pt_pool.tile([P, D], BF16)
                for dc in range(NDC):
                    nc.tensor.transpose(
                        pt_all[:, dc * P:(dc + 1) * P],
                        acc[:, dc * P:(dc + 1) * P],
                        ident[:],
                    )
                return pt_all

            def project_core(pt_all):
                # PSUM->SBUF chunk copies on DVE (cheap there, and emitted
                # mid-way through the NEXT block's adds so the transposes
                # they wait on are already done), then the K-projection on
                # PE and tanh on ACT. The y write is emitted separately so
                # it can sit BEHIND the next block's load triggers in the
                # sync ring FIFO (never head-blocking the x stream).
                xt_all = tpool.tile([P, D], BF16)
                for dc in range(NDC):
                    nc.vector.tensor_copy(
                        out=xt_all[:, dc * P:(dc + 1) * P],
                        in_=pt_all[:, dc * P:(dc + 1) * P],
                    )

                po = po_pool.tile([P, K], F32)
                # bias broadcast across partitions: ones[1,128].T @ bias[1,64]
                nc.tensor.matmul(
                    po[:], lhsT=ones_sb[:], rhs=bias_sb[:], start=True, stop=False
                )
                for dc in range(NDC):
                    nc.tensor.matmul(
                        po[:],
                        lhsT=xt_all[:, dc * P:(dc + 1) * P],
                        rhs=wt_sb[:, dc * K:(dc + 1) * K],
                        start=False,
                        stop=(dc == NDC - 1),
                    )
                ot = opool.tile([P, K], F32)
                nc.scalar.activation(
                    ot[:], po[:], mybir.ActivationFunctionType.Tanh
                )
                return ot

            def write_y(blk, ot):
                b0 = blk * P
                nc.sync.dma_start(out=yap[b0:b0 + P, :], in_=ot[:])

            def tail_block(blk, xt, mid_cb):
                # last block: same arithmetic as reduce+transpose+project,
                # but the post-arrival chain is split into column halves so
                # DVE adds on the hi half overlap PE transposes on the lo
                # half, shaving ~3us off the kernel tail
                def addf(i0, i1):
                    t = ppool.tile([P, D], BF16)
                    nc.vector.tensor_add(out=t[:], in0=i0[:], in1=i1[:])
                    return t

                ae = addf(xt[0], xt[2])
                ao = addf(xt[1], xt[3])
                be = addf(xt[4], xt[6])
                bo = addf(xt[5], xt[7])
                mid_cb()
                ce = addf(ae, be)
                co = addf(ao, bo)

                H = D // 2
                NH = H // P
                pt_all = pt_pool.tile([P, D], BF16)
                xt_all = tpool.tile([P, D], BF16)
                po = po_pool.tile([P, K], F32)
                for h in range(2):
                    sl = slice(h * H, (h + 1) * H)

                    def addh(a0, s0, a1, s1):
                        t = ppool.tile([P, H], BF16)
                        nc.vector.tensor_add(out=t[:], in0=a0[s0], in1=a1[s1])
                        return t

                    cfull = (slice(None), sl)
                    hfull = (slice(None), slice(None))
                    eh = addh(xt[8], cfull, xt[10], cfull)
                    oh = addh(xt[9], cfull, xt[11], cfull)
                    fh = addh(ce, cfull, eh, hfull)
                    gh = addh(co, cfull, oh, hfull)
                    ah = addh(fh, hfull, gh, hfull)
                    for dcl in range(NH):
                        dc = h * NH + dcl
                        nc.tensor.transpose(
                            pt_all[:, dc * P:(dc + 1) * P],
                            ah[:, dcl * P:(dcl + 1) * P],
                            ident[:],
                        )
                nc.tensor.matmul(
                    po[:], lhsT=ones_sb[:], rhs=bias_sb[:], start=True, stop=False
                )
                for h in range(2):
                    for dcl in range(NH):
                        dc = h * NH + dcl
                        nc.vector.tensor_copy(
                            out=xt_all[:, dc * P:(dc + 1) * P],
                            in_=pt_all[:, dc * P:(dc + 1) * P],
                        )
                    for dcl in range(NH):
                        dc = h * NH + dcl
                        nc.tensor.matmul(
                            po[:],
                            lhsT=xt_all[:, dc * P:(dc + 1) * P],
                            rhs=wt_sb[:, dc * K:(dc + 1) * K],
                            start=False,
                            stop=(dc == NDC - 1),
                        )
                ot = opool.tile([P, K], F32)
                nc.scalar.activation(
                    ot[:], po[:], mybir.ActivationFunctionType.Tanh
                )
                write_y(blk, ot)

            # Software pipeline per iteration blk:
            #   adds(blk) [DVE, with copies+matmuls+tanh of blk-1 emitted
            #   after the 4th add] -> loads(blk+1) triggers [sync/scalar] ->
            #   transposes(blk) [PE]. No engine's instruction stream ever
            #   waits across the block boundary: the rings see only triggers,
            #   DVE's mid-block detour only touches work that is already
            #   complete, and PE follows DVE by less than a block.
            xt = issue_loads(0)
            prev = None
            for blk in range(NBLK - 1):
                if prev is None:
                    acc = reduce(xt)
                    got = None
                else:
                    pblk, ppt = prev
                    got = {}
                    acc = reduce(
                        xt,
                        mid_cb=lambda: got.__setitem__(
                            "ot", project_core(ppt)
                        ),
                    )
                xt = issue_loads(blk + 1)
                if got is not None:
                    write_y(pblk, got["ot"])
                prev = (blk, transpose_block(acc))
            pblk, ppt = prev

            def tail_mid():
                write_y(pblk, project_core(ppt))

            tail_block(NBLK - 1, xt, mid_cb=tail_mid)

    nc.compile()
    _nc_cache = nc
    return nc


def _ensure_ntff_hook():
    """Register the axon NTFF profile hook if the image's antenv lacks it."""
    import sys
    import types

    try:
        from antenv.axon_hooks import get_axon_ntff_profile_hook  # noqa: F401
        return
    except ImportError:
        pass
    import antenv

    mod = types.ModuleType("antenv.axon_hooks")
    mod._hook = None

    def set_axon_ntff_profile_hook(h):
        mod._hook = h

    def get_axon_ntff_profile_hook():
        return mod._hook

    mod.set_axon_ntff_profile_hook = set_axon_ntff_profile_hook
    mod.get_axon_ntff_profile_hook = get_axon_ntff_profile_hook
    sys.modules["antenv.axon_hooks"] = mod
    antenv.axon_hooks = mod
    try:
        from trn_agent_boot.trn_boot import _ntff_profile_via_ctypes

        mod._hook = _ntff_profile_via_ctypes("/opt/axon/libaxon_pjrt.so")
    except Exception:
        mod._hook = None


def _run(inputs, trace=False, **kwargs):
    import ml_dtypes

    bf16 = np.dtype(ml_dtypes.bfloat16)
    x = np.asarray(inputs["x"], dtype=np.float32)
    W = np.asarray(inputs["W"], dtype=np.float32)
    b = np.asarray(inputs["b"], dtype=np.float32)
    wt = (np.ascontiguousarray(W.T) * np.float32(1.0 / L)).astype(bf16)
    bias = np.ascontiguousarray(b.reshape(1, K)).astype(bf16)
    in_maps = [
        {
            "x": np.ascontiguousarray(x[:, c * BS:(c + 1) * BS, :]),
            "wt": wt,
            "bias": bias,
        }
        for c in range(NCORES)
    ]
    if trace:
        _ensure_ntff_hook()
        import concourse.bass_utils as bu

        bu.upload_artifacts = lambda tmpdir: "local://skipped"
    nc = _build()
    res = run_bass_kernel_spmd(
        nc, in_maps, core_ids=list(range(NCORES)), trace=trace, **kwargs
    )
    y = np.concatenate([r["y"] for r in res.results], axis=0)
    return y, res


def kernel(**inputs):
    y, _ = _run(inputs)
    return y
